# revision 1
# baseline (speedup 1.0000x reference)
"""CommNet forward on 8 TRN2 NeuronCores (Bass/Tile).

Model (per reference):
    h0 = emb[agent_ids]                      # (B, M, H)
    repeat 4x:
        c = (sum_m h - h) / (M-1)
        x = [h, c, h0]                       # (B, M, 3H)
        d = relu(x @ W1 + b1) @ W2 + b2
        h = h + d
    logits = h @ Wd + bd                     # (B, M, A)

Constants: B=1024, M=64, H=256, A=16, V=1000, 4 comm steps.

Sharding: data-parallel on B across 8 cores (128 groups / core); weights
replicated. Within a core every tensor is laid out [hidden-on-partitions,
tokens-on-free] (tokens = group*64 + agent, T=8192 per core).

Algebra used on-device (host folds weights accordingly):
    x @ W1 = h @ (W1h - inv*W1c) + S @ (inv*W1c) + h0 @ W1h0
with S = sum_m h broadcast per group, inv = 1/(M-1).  z0 = h0 @ W1h0 + b1 is
precomputed once; per step the PSUM accumulation is:
    psum1 = W1hp.T@h(K=2) + I.T@z0b + I.T@bcast(SW)      -> d1 = relu(psum1)
    psum2 = W2.T@d1(K=2)  + I.T@h                        -> h  = psum2 + b2
Matmuls run as float32r (fp32 bits, tf32-class rounding, 1 cyc/row).
"""

import numpy as np

B, M, H, A, V = 1024, 64, 256, 16, 1000
STEPS = 4
NCORES = 8
G = B // NCORES          # groups per core = 128
T = G * M                # tokens per core = 8192
P = 128                  # partitions
KT = H // P              # K tiles per H = 2
NCH = T // 512           # 512-token chunks = 16
CH = 512
GPC = CH // M            # groups per chunk = 8
INV = 1.0 / (M - 1)

_CACHE = {}


def _build():
    import concourse.bass as bass
    import concourse.tile as tile
    from concourse import bacc, mybir
    from concourse.masks import make_identity

    F32 = mybir.dt.float32
    F32R = mybir.dt.float32r
    I32 = mybir.dt.int32

    nc = bacc.Bacc("TRN2", target_bir_lowering=False, debug=False,
                   num_devices=NCORES)

    ids_d = nc.dram_tensor("ids_pt", [P, T // P], I32, kind="ExternalInput").ap()
    emb_d = nc.dram_tensor("emb", [V, H], F32, kind="ExternalInput").ap()
    w1hp_d = nc.dram_tensor("w1hp", [P, KT, H], F32, kind="ExternalInput").ap()
    w1h0_d = nc.dram_tensor("w1h0", [P, KT, H], F32, kind="ExternalInput").ap()
    w1ci_d = nc.dram_tensor("w1ci", [P, KT, H], F32, kind="ExternalInput").ap()
    w2_d = nc.dram_tensor("w2", [P, KT, H], F32, kind="ExternalInput").ap()
    wd_d = nc.dram_tensor("wd", [P, KT, A], F32, kind="ExternalInput").ap()
    b1_d = nc.dram_tensor("b1p", [P, KT], F32, kind="ExternalInput").ap()
    bs_d = nc.dram_tensor("bsteps", [P, KT, STEPS], F32, kind="ExternalInput").ap()
    bd_d = nc.dram_tensor("bdp", [A, 1], F32, kind="ExternalInput").ap()
    logT_d = nc.dram_tensor("logT", [A, T], F32, kind="ExternalOutput").ap()

    with tile.TileContext(nc) as tc:
        with (
            tc.tile_pool(name="const", bufs=1) as const,
            tc.tile_pool(name="big", bufs=1) as big,
            tc.tile_pool(name="stage", bufs=2) as stage,
            tc.tile_pool(name="gat", bufs=4) as gat,
            tc.tile_pool(name="d1p", bufs=2) as d1p,
            tc.tile_pool(name="swp", bufs=2) as swp,
            tc.tile_pool(name="lgt", bufs=2) as lgt,
            tc.tile_pool(name="ps", bufs=2, space="PSUM") as ps,
        ):
            # ---- constants / weights -------------------------------------
            ids = const.tile([P, T // P], I32)
            nc.sync.dma_start(out=ids[:], in_=ids_d[:])

            ident = const.tile([P, P], F32)
            make_identity(nc, ident[:])
            ident_r = const.tile([P, P], F32R)
            nc.vector.tensor_copy(ident_r[:], ident[:])

            w1ci = const.tile([P, KT, H], F32)
            nc.sync.dma_start(out=w1ci[:], in_=w1ci_d[:])
            b1 = const.tile([P, KT], F32)
            nc.sync.dma_start(out=b1[:], in_=b1_d[:])
            bsteps = const.tile([P, KT, STEPS], F32)
            nc.sync.dma_start(out=bsteps[:], in_=bs_d[:])
            bd = const.tile([A, 1], F32)
            nc.sync.dma_start(out=bd[:], in_=bd_d[:])

            def load_r(dram, shape, name):
                st = stage.tile(shape, F32, tag="wstage", name=f"st_{name}")
                nc.sync.dma_start(out=st[:], in_=dram)
                t = const.tile(shape, F32R, tag=name, name=name)
                nc.vector.tensor_copy(t[:], st[:])
                return t

            w1hp = load_r(w1hp_d[:], [P, KT, H], "w1hp_r")
            w1h0 = load_r(w1h0_d[:], [P, KT, H], "w1h0_r")
            w2 = load_r(w2_d[:], [P, KT, H], "w2_r")
            wd = load_r(wd_d[:], [P, KT, A], "wd_r")

            # ---- big state tiles -----------------------------------------
            h = [big.tile([P, T], F32R, tag=f"h{k}", name=f"h{k}") for k in range(KT)]
            z0b = [big.tile([P, T], F32R, tag=f"z0b{k}", name=f"z0b{k}") for k in range(KT)]
            # S double-buffered across steps: S[par][k] [P, G] fp32
            S = [[big.tile([P, G], F32, tag=f"S{par}{k}", name=f"S{par}{k}") for k in range(KT)]
                 for par in range(2)]

            # ---- phase 1: gather h0 rows, transpose into h ---------------
            for t in range(T // P):
                gst = gat.tile([P, H], F32, tag="gst")
                nc.gpsimd.indirect_dma_start(
                    out=gst[:],
                    out_offset=None,
                    in_=emb_d[:],
                    in_offset=bass.IndirectOffsetOnAxis(ap=ids[:, t:t + 1], axis=0),
                )
                for k in range(KT):
                    pt = ps.tile([P, P], F32, space="PSUM", tag=f"mm1_{k}")
                    nc.tensor.transpose(out=pt[:], in_=gst[:, k * P:(k + 1) * P],
                                        identity=ident[:])
                    if (t + k) % 2 == 0:
                        nc.vector.tensor_copy(h[k][:, t * P:(t + 1) * P], pt[:])
                    else:
                        nc.scalar.activation(
                            out=h[k][:, t * P:(t + 1) * P], in_=pt[:],
                            func=mybir.ActivationFunctionType.Identity)

            # ---- phase 2: z0b = h0 @ W1h0 + b1 ; S0 = segsum(h0) ---------
            for q in range(NCH):
                qs = slice(q * CH, (q + 1) * CH)
                for j in range(KT):
                    pz = ps.tile([P, CH], F32, space="PSUM", tag=f"mm2_{j}")
                    for k in range(KT):
                        nc.tensor.matmul(
                            pz[:], w1h0[:, k, j * P:(j + 1) * P], h[k][:, qs],
                            start=(k == 0), stop=(k == KT - 1))
                    nc.scalar.activation(
                        out=z0b[j][:, qs], in_=pz[:],
                        func=mybir.ActivationFunctionType.Identity,
                        bias=b1[:, j:j + 1])
                gq = slice(q * GPC, (q + 1) * GPC)
                for k in range(KT):
                    nc.vector.tensor_reduce(
                        out=S[0][k][:, gq],
                        in_=h[k][:, qs].bitcast(F32).rearrange(
                            "p (g m) -> p g m", m=M),
                        axis=mybir.AxisListType.X, op=mybir.AluOpType.add)

            # ---- phase 3: comm steps -------------------------------------
            for s in range(STEPS):
                Scur, Snxt = S[s % 2], S[(s + 1) % 2]
                last = s == STEPS - 1
                HQ = NCH // 2  # chunks per group-half
                HLF = G // 2   # groups per half
                swsb = None
                for q in range(NCH):
                    if q % HQ == 0:
                        # SW for this half: (inv*W1c).T @ S[:, half]  [P, HLF]
                        # Computed per half so step s can start its first
                        # chunks before step s-1 finished its second half.
                        half = q // HQ
                        hs = slice(half * HLF, (half + 1) * HLF)
                        swsb = []
                        for j in range(KT):
                            psw = ps.tile([P, HLF], F32, space="PSUM",
                                          tag=f"mm1_{j}", name="psw")
                            for k in range(KT):
                                nc.tensor.matmul(
                                    psw[:], w1ci[:, k, j * P:(j + 1) * P],
                                    Scur[k][:, hs],
                                    start=(k == 0), stop=(k == KT - 1))
                            sw = swp.tile([P, HLF], F32R,
                                          tag=f"sw{half}{j}", name="sw")
                            nc.vector.tensor_copy(sw[:], psw[:])
                            swsb.append(sw)
                    qs = slice(q * CH, (q + 1) * CH)
                    gq = slice(q * GPC, (q + 1) * GPC)
                    lgq = slice((q % HQ) * GPC, (q % HQ + 1) * GPC)
                    d1 = []
                    for j in range(KT):
                        p1 = ps.tile([P, CH], F32, space="PSUM", tag=f"mm1_{j}")
                        for k in range(KT):
                            nc.tensor.matmul(
                                p1[:], w1hp[:, k, j * P:(j + 1) * P], h[k][:, qs],
                                start=(k == 0), stop=False)
                        nc.tensor.matmul(p1[:], ident_r[:], z0b[j][:, qs],
                                         start=False, stop=False)
                        nc.tensor.matmul(
                            p1[:].rearrange("p (g m) -> p g m", g=GPC),
                            ident_r[:],
                            swsb[j][:, lgq].to_broadcast([P, GPC, M]),
                            start=False, stop=True)
                        d = d1p.tile([P, CH], F32R, tag=f"d1_{j}")
                        nc.scalar.activation(
                            out=d[:], in_=p1[:],
                            func=mybir.ActivationFunctionType.Relu,
                            bias=bsteps[:, j, s:s + 1])
                        d1.append(d)
                    for j in range(KT):
                        p2 = ps.tile([P, CH], F32, space="PSUM", tag=f"mm2_{j}")
                        for k in range(KT):
                            nc.tensor.matmul(
                                p2[:], w2[:, k, j * P:(j + 1) * P], d1[k][:],
                                start=(k == 0), stop=(k == KT - 1))
                        nc.vector.tensor_add(
                            h[j][:, qs], h[j][:, qs].bitcast(F32), p2[:])
                        if not last:
                            nc.vector.tensor_reduce(
                                out=Snxt[j][:, gq],
                                in_=h[j][:, qs].bitcast(F32).rearrange(
                                    "p (g m) -> p g m", m=M),
                                axis=mybir.AxisListType.X,
                                op=mybir.AluOpType.add)
                    if last:
                        # logits for this chunk
                        pl = ps.tile([A, CH], F32, space="PSUM", tag="mm2_0",
                                     name="pl")
                        for k in range(KT):
                            nc.tensor.matmul(pl[:], wd[:, k, :], h[k][:, qs],
                                             start=(k == 0), stop=(k == KT - 1))
                        lg = lgt.tile([A, CH], F32, tag="lg")
                        nc.scalar.activation(
                            out=lg[:], in_=pl[:],
                            func=mybir.ActivationFunctionType.Identity,
                            bias=bd[:, 0:1])
                        nc.sync.dma_start(out=logT_d[:, qs], in_=lg[:])

    nc.compile()
    return nc


def _prep_inputs(agent_ids, emb, W1, b1, W2, b2, Wd, bd):
    agent_ids = np.asarray(agent_ids)
    emb = np.ascontiguousarray(np.asarray(emb, dtype=np.float32))
    W1 = np.asarray(W1, dtype=np.float32)
    b1 = np.asarray(b1, dtype=np.float32)
    W2 = np.asarray(W2, dtype=np.float32)
    b2 = np.asarray(b2, dtype=np.float32)
    Wd = np.asarray(Wd, dtype=np.float32)
    bd = np.asarray(bd, dtype=np.float32)

    W1h, W1c, W1h0 = W1[:H], W1[H:2 * H], W1[2 * H:]
    w1hp = W1h - INV * W1c
    w1ci = INV * W1c
    # b2 is never added on device: h' tracks h - s*b2.  Its effect on the
    # step-s pre-activation is s * b2 @ (W1h + W1c); on logits, 4 * b2 @ Wd.
    bb = b2 @ (W1h + W1c)
    bsteps = np.stack([s * bb for s in range(STEPS)], axis=1)  # [H, STEPS]
    bdp = bd + STEPS * (b2 @ Wd)

    def pack(w):  # [H, out] -> [P, KT, out]
        return np.ascontiguousarray(
            w.reshape(KT, P, w.shape[1]).transpose(1, 0, 2))

    def packb(b):  # [H] -> [P, KT]
        return np.ascontiguousarray(b.reshape(KT, P).T)

    shared = {
        "emb": emb,
        "w1hp": pack(w1hp),
        "w1h0": pack(W1h0),
        "w1ci": pack(w1ci),
        "w2": pack(W2),
        "wd": pack(Wd),
        "b1p": packb(b1),
        "bsteps": np.ascontiguousarray(
            bsteps.reshape(KT, P, STEPS).transpose(1, 0, 2)),
        "bdp": np.ascontiguousarray(bdp.reshape(A, 1)),
    }
    in_maps = []
    for c in range(NCORES):
        ids_local = np.asarray(
            agent_ids[c * G:(c + 1) * G], dtype=np.int32).reshape(T)
        ids_pt = np.ascontiguousarray(ids_local.reshape(T // P, P).T)
        in_maps.append({"ids_pt": ids_pt, **shared})
    return in_maps


def _run(in_maps, trace=False, tmpdir=None):
    from concourse.bass_utils import run_bass_kernel_spmd

    if "nc" not in _CACHE:
        _CACHE["nc"] = _build()
    nc = _CACHE["nc"]
    res = run_bass_kernel_spmd(
        nc, in_maps, core_ids=list(range(NCORES)), trace=trace, tmpdir=tmpdir)
    out = np.empty((B, M, A), dtype=np.float32)
    for c in range(NCORES):
        logT = res.results[c]["logT"]  # [A, T]
        out[c * G:(c + 1) * G] = logT.T.reshape(G, M, A)
    return out, res


def kernel(agent_ids, emb, W1, b1, W2, b2, Wd, bd):
    in_maps = _prep_inputs(agent_ids, emb, W1, b1, W2, b2, Wd, bd)
    out, _ = _run(in_maps, trace=False)
    return out



# revision 5
# speedup vs baseline: 1.0567x; 1.0567x over previous
"""CommNet forward on 8 TRN2 NeuronCores (Bass/Tile) — v2 bf16.

Model (per reference):
    h0 = emb[agent_ids]                      # (B, M, H)
    repeat 4x:
        c = (sum_m h - h) / (M-1)
        x = [h, c, h0]                       # (B, M, 3H)
        d = relu(x @ W1 + b1) @ W2 + b2
        h = h + d
    logits = h @ Wd + bd                     # (B, M, A)

Constants: B=1024, M=64, H=256, A=16, V=1000, 4 comm steps.

Sharding: data-parallel on B across 8 cores (128 groups / core); weights
replicated. Per core every tensor is [hidden-on-partitions, tokens-on-free]
(tokens = group*64 + agent, T=8192).

v2 changes vs v1:
  * All matmul operands bf16 (FWL weight loads, 2x DVE); PSUM stays fp32.
    Predicted end-to-end rel err ~3.5e-3 (gate 2e-2).
  * Embedding gather via batched dma_gather(transpose=True): lands h0
    directly in [P, KT, tokens] layout — no PE transposes, no per-row
    indirect DMAs (64x 1us fixed overhead -> 4x ~1.7us).
  * z0b + broadcast(SW) folded into one tile E on GpSimd, added into the
    W1 PSUM with a single identity matmul (5 MMs per chunk,j instead of 6).

Algebra on device (host folds weights):
    x @ W1 = h @ (W1h - inv*W1c) + S @ (inv*W1c) + h0 @ W1h0
with S = sum_m h per group, inv = 1/(M-1).  z0b = h0 @ W1h0 + b1 computed
once; b2 never added on device (h tracks h - s*b2, corrected via bsteps
bias and bdp at logits).
"""

import numpy as np

B, M, H, A, V = 1024, 64, 256, 16, 1000
STEPS = 4
NCORES = 8
G = B // NCORES          # groups per core = 128
T = G * M                # tokens per core = 8192
P = 128                  # partitions
KT = H // P              # K tiles per H = 2
NCH = T // 512           # 512-token chunks = 16
CH = 512
GPC = CH // M            # groups per chunk = 8
INV = 1.0 / (M - 1)
NG = NCH                 # gather calls (one per 512-token chunk;
GI = T // NG             # transpose-mode dma_gather caps at 512 idxs)
CPG = GI // CH           # chunks per gather call = 1

_CACHE = {}


def _build():
    import concourse.bass as bass
    import concourse.tile as tile
    from concourse import bacc, mybir
    from concourse.masks import make_identity

    F32 = mybir.dt.float32
    F32R = mybir.dt.float32r
    BF16 = mybir.dt.bfloat16
    I16 = mybir.dt.int16

    nc = bacc.Bacc("TRN2", target_bir_lowering=False, debug=False,
                   num_devices=NCORES)

    idx_d = nc.dram_tensor("idx16", [P, T // 16], I16, kind="ExternalInput").ap()
    emb_d = nc.dram_tensor("embb", [V, H], BF16, kind="ExternalInput").ap()
    w1hp_d = nc.dram_tensor("w1hp", [P, KT, H], BF16, kind="ExternalInput").ap()
    w1h0_d = nc.dram_tensor("w1h0", [P, KT, H], BF16, kind="ExternalInput").ap()
    w1ci_d = nc.dram_tensor("w1ci", [P, KT, H], F32, kind="ExternalInput").ap()
    w2_d = nc.dram_tensor("w2", [P, KT, H], BF16, kind="ExternalInput").ap()
    wd_d = nc.dram_tensor("wd", [P, KT, A], BF16, kind="ExternalInput").ap()
    b1_d = nc.dram_tensor("b1p", [P, KT], F32, kind="ExternalInput").ap()
    bs_d = nc.dram_tensor("bsteps", [P, KT, STEPS], F32, kind="ExternalInput").ap()
    bd_d = nc.dram_tensor("bdp", [A, 1], F32, kind="ExternalInput").ap()
    logT_d = nc.dram_tensor("logT", [A, T], F32, kind="ExternalOutput").ap()

    with tile.TileContext(nc) as tc:
        with (
            tc.tile_pool(name="const", bufs=1) as const,
            tc.tile_pool(name="big", bufs=1) as big,
            tc.tile_pool(name="d1p", bufs=2) as d1p,
            tc.tile_pool(name="ep", bufs=3) as ep,
            tc.tile_pool(name="swp", bufs=2) as swp,
            tc.tile_pool(name="lgt", bufs=2) as lgt,
            tc.tile_pool(name="ps", bufs=2, space="PSUM") as ps,
        ):
            # ---- constants / weights -------------------------------------
            idx = const.tile([P, T // 16], I16)
            nc.sync.dma_start(out=idx[:], in_=idx_d[:])

            ident = const.tile([P, P], BF16)
            make_identity(nc, ident[:])

            w1hp = const.tile([P, KT, H], BF16)
            nc.sync.dma_start(out=w1hp[:], in_=w1hp_d[:])
            w1h0 = const.tile([P, KT, H], BF16)
            nc.sync.dma_start(out=w1h0[:], in_=w1h0_d[:])
            w2 = const.tile([P, KT, H], BF16)
            nc.sync.dma_start(out=w2[:], in_=w2_d[:])
            wd = const.tile([P, KT, A], BF16)
            nc.sync.dma_start(out=wd[:], in_=wd_d[:])
            w1ci = const.tile([P, KT, H], F32)
            nc.sync.dma_start(out=w1ci[:], in_=w1ci_d[:])
            b1 = const.tile([P, KT], F32)
            nc.sync.dma_start(out=b1[:], in_=b1_d[:])
            bsteps = const.tile([P, KT, STEPS], F32)
            nc.sync.dma_start(out=bsteps[:], in_=bs_d[:])
            bd = const.tile([A, 1], F32)
            nc.sync.dma_start(out=bd[:], in_=bd_d[:])

            # ---- state tiles ---------------------------------------------
            # h storage: one tile per gather call, [P, KT, GI] bf16
            hh = [big.tile([P, KT, GI], BF16, tag=f"hh{c}", name=f"hh{c}")
                  for c in range(NG)]
            zb = [big.tile([P, T], BF16, tag=f"zb{k}", name=f"zb{k}")
                  for k in range(KT)]
            S = [[big.tile([P, G], F32, tag=f"S{par}{k}", name=f"S{par}{k}")
                  for k in range(KT)] for par in range(2)]

            def hv(k, q):
                """h[k] view for chunk q: [P, CH] bf16."""
                c, lo = q // CPG, (q % CPG) * CH
                return hh[c][:, k, lo:lo + CH]

            # ---- phase 1: gather h0 (pre-transposed) ---------------------
            for c in range(NG):
                nc.gpsimd.dma_gather(
                    out_ap=hh[c][:],
                    in_ap=emb_d[:],
                    idxs_ap=idx[:, c * (GI // 16):(c + 1) * (GI // 16)],
                    num_idxs=GI,
                    num_idxs_reg=GI,
                    elem_size=H,
                    transpose=True,
                    queue_num=0,
                )

            # ---- phase 2: z0b = h0 @ W1h0 + b1 ; S0 = segsum(h0) ---------
            for q in range(NCH):
                qs = slice(q * CH, (q + 1) * CH)
                for j in range(KT):
                    pz = ps.tile([P, CH], F32, space="PSUM", tag=f"mm1_{j}")
                    for k in range(KT):
                        nc.tensor.matmul(
                            pz[:], w1h0[:, k, j * P:(j + 1) * P], hv(k, q),
                            start=(k == 0), stop=(k == KT - 1))
                    nc.scalar.activation(
                        out=zb[j][:, qs], in_=pz[:],
                        func=mybir.ActivationFunctionType.Identity,
                        bias=b1[:, j:j + 1])
                gq = slice(q * GPC, (q + 1) * GPC)
                for k in range(KT):
                    nc.vector.tensor_reduce(
                        out=S[0][k][:, gq],
                        in_=hv(k, q).rearrange("p (g m) -> p g m", m=M),
                        axis=mybir.AxisListType.X, op=mybir.AluOpType.add)

            # ---- phase 3: comm steps -------------------------------------
            for s in range(STEPS):
                Scur, Snxt = S[s % 2], S[(s + 1) % 2]
                last = s == STEPS - 1
                HQ = NCH // 2  # chunks per group-half
                HLF = G // 2   # groups per half
                swsb = None
                for q in range(NCH):
                    if q % HQ == 0:
                        # SW for this half: (inv*W1c).T @ S[:, half]  [P, HLF]
                        half = q // HQ
                        hs = slice(half * HLF, (half + 1) * HLF)
                        swsb = []
                        for j in range(KT):
                            psw = ps.tile([P, HLF], F32, space="PSUM",
                                          tag=f"mm1_{j}", name="psw")
                            for k in range(KT):
                                nc.tensor.matmul(
                                    psw[:],
                                    w1ci[:, k, j * P:(j + 1) * P],
                                    Scur[k][:, hs],
                                    start=(k == 0), stop=(k == KT - 1))
                            sw = swp.tile([P, HLF], BF16,
                                          tag=f"sw{half}{j}", name="sw")
                            nc.vector.tensor_copy(sw[:], psw[:])
                            swsb.append(sw)
                    qs = slice(q * CH, (q + 1) * CH)
                    gq = slice(q * GPC, (q + 1) * GPC)
                    lgq = slice((q % HQ) * GPC, (q % HQ + 1) * GPC)
                    # E_j = z0b_j + broadcast(sw_j)  (GpSimd, SBUF only)
                    Es = []
                    for j in range(KT):
                        E = ep.tile([P, CH], BF16, tag=f"e{j}", name=f"E{j}")
                        nc.gpsimd.tensor_tensor(
                            out=E[:].rearrange("p (g m) -> p g m", m=M),
                            in0=zb[j][:, qs].rearrange("p (g m) -> p g m", m=M),
                            in1=swsb[j][:, lgq].to_broadcast([P, GPC, M]),
                            op=mybir.AluOpType.add)
                        Es.append(E)
                    d1 = []
                    for j in range(KT):
                        p1 = ps.tile([P, CH], F32, space="PSUM", tag=f"mm1_{j}")
                        for k in range(KT):
                            nc.tensor.matmul(
                                p1[:], w1hp[:, k, j * P:(j + 1) * P], hv(k, q),
                                start=(k == 0), stop=False)
                        nc.tensor.matmul(p1[:], ident[:], Es[j][:],
                                         start=False, stop=True)
                        d = d1p.tile([P, CH], BF16, tag=f"d1_{j}")
                        nc.scalar.activation(
                            out=d[:], in_=p1[:],
                            func=mybir.ActivationFunctionType.Relu,
                            bias=bsteps[:, j, s:s + 1])
                        d1.append(d)
                    for j in range(KT):
                        p2 = ps.tile([P, CH], F32, space="PSUM", tag=f"mm2_{j}")
                        for k in range(KT):
                            nc.tensor.matmul(
                                p2[:], w2[:, k, j * P:(j + 1) * P], d1[k][:],
                                start=(k == 0), stop=(k == KT - 1))
                        nc.vector.tensor_add(hv(j, q), hv(j, q), p2[:])
                        if not last:
                            nc.vector.tensor_reduce(
                                out=Snxt[j][:, gq],
                                in_=hv(j, q).rearrange("p (g m) -> p g m", m=M),
                                axis=mybir.AxisListType.X,
                                op=mybir.AluOpType.add)
                    if last:
                        pl = ps.tile([A, CH], F32, space="PSUM", tag="mm2_0",
                                     name="pl")
                        for k in range(KT):
                            nc.tensor.matmul(pl[:], wd[:, k, :], hv(k, q),
                                             start=(k == 0), stop=(k == KT - 1))
                        lg = lgt.tile([A, CH], F32, tag="lg")
                        nc.scalar.activation(
                            out=lg[:], in_=pl[:],
                            func=mybir.ActivationFunctionType.Identity,
                            bias=bd[:, 0:1])
                        nc.sync.dma_start(out=logT_d[:, qs], in_=lg[:])

    nc.compile()
    return nc


def _to_bf16(x):
    import ml_dtypes
    return np.ascontiguousarray(x.astype(ml_dtypes.bfloat16))


def _prep_inputs(agent_ids, emb, W1, b1, W2, b2, Wd, bd):
    agent_ids = np.asarray(agent_ids)
    emb = np.asarray(emb, dtype=np.float32)
    W1 = np.asarray(W1, dtype=np.float32)
    b1 = np.asarray(b1, dtype=np.float32)
    W2 = np.asarray(W2, dtype=np.float32)
    b2 = np.asarray(b2, dtype=np.float32)
    Wd = np.asarray(Wd, dtype=np.float32)
    bd = np.asarray(bd, dtype=np.float32)

    W1h, W1c, W1h0 = W1[:H], W1[H:2 * H], W1[2 * H:]
    w1hp = W1h - INV * W1c
    w1ci = INV * W1c
    # b2 never added on device: h tracks h - s*b2.  Correction to the step-s
    # pre-activation is s * b2 @ (W1h + W1c); at logits, 4 * b2 @ Wd.
    bb = b2 @ (W1h + W1c)
    bsteps = np.stack([s * bb for s in range(STEPS)], axis=1)  # [H, STEPS]
    bdp = bd + STEPS * (b2 @ Wd)

    def pack(w, dt=None):  # [H, out] -> [P, KT, out]
        r = np.ascontiguousarray(w.reshape(KT, P, w.shape[1]).transpose(1, 0, 2))
        return _to_bf16(r) if dt == "bf16" else r

    def packb(b):  # [H] -> [P, KT]
        return np.ascontiguousarray(b.reshape(KT, P).T)

    shared = {
        "embb": _to_bf16(emb),
        "w1hp": pack(w1hp, "bf16"),
        "w1h0": pack(W1h0, "bf16"),
        "w1ci": pack(w1ci),
        "w2": pack(W2, "bf16"),
        "wd": pack(Wd, "bf16"),
        "b1p": packb(b1),
        "bsteps": np.ascontiguousarray(
            bsteps.reshape(KT, P, STEPS).transpose(1, 0, 2)),
        "bdp": np.ascontiguousarray(bdp.reshape(A, 1)),
    }
    in_maps = []
    for c in range(NCORES):
        ids_local = np.asarray(
            agent_ids[c * G:(c + 1) * G], dtype=np.int16).reshape(T)
        # dma_gather wrap: idx i -> partition i%16, slot i//16; replicated
        # across the 8 Q7 cores (16 partitions each).
        wrapped = np.ascontiguousarray(ids_local.reshape(T // 16, 16).T)
        idx128 = np.ascontiguousarray(np.tile(wrapped, (8, 1)))
        in_maps.append({"idx16": idx128, **shared})
    return in_maps


def _run(in_maps, trace=False, tmpdir=None):
    from concourse.bass_utils import run_bass_kernel_spmd

    if "nc" not in _CACHE:
        _CACHE["nc"] = _build()
    nc = _CACHE["nc"]
    res = run_bass_kernel_spmd(
        nc, in_maps, core_ids=list(range(NCORES)), trace=trace, tmpdir=tmpdir)
    out = np.empty((B, M, A), dtype=np.float32)
    for c in range(NCORES):
        logT = res.results[c]["logT"]  # [A, T]
        out[c * G:(c + 1) * G] = logT.T.reshape(G, M, A)
    return out, res


def kernel(agent_ids, emb, W1, b1, W2, b2, Wd, bd):
    in_maps = _prep_inputs(agent_ids, emb, W1, b1, W2, b2, Wd, bd)
    out, _ = _run(in_maps, trace=False)
    return out
